# revision 20
# baseline (speedup 1.0000x reference)
"""Trainium2 Bass kernel for GCN(1->8) + flatten + big regression matvec.

Model (reference):
    h = GCNConv(x[4096,1], edge_index[2,131072], W1[1,8], b1[8])   # [4096, 8]
    h = relu(h.reshape(-1))                                        # [32768]
    y = h @ Wr[32768, 4096] + br                                   # [4096]

Since x is [N,1] and W1 is [1,8], the GCN collapses to a per-node scalar
    s[d] = dinv[d] * sum_s C'[d, s] * u[s],   u = x * dinv,
    dinv = 1/sqrt(1 + indeg),   C' = edge-count matrix + I,
and h[d,k] = relu(s[d]*W1[k] + b1[k]).

Sharding: row-parallel (contraction) split of the matvec across 8 cores.
Core k owns nodes [512k, 512k+512) and the matching 4096 rows of Wr
(shipped as bf16; ~0.2% output error, well inside tolerance).  The
message passing is a dense matmul against the core's [512, 4096] slice of
C' (fp8e4m3, exact for integer counts <= 8, bf16 fallback otherwise),
with u split into three scaled fp8 terms (u = p0 + p1/64 + p2/4096) so
the aggregation is fp32-accurate.  dinv is computed on device with ACT
Sqrt + DVE reciprocal + two Newton iterations.  br is preloaded into the
PSUM accumulators (so the matvec adds it for free) on core 0 only.  Each
core emits a partial y[4096]; the host sums the 8 partials.

The node grid on each core is column-rotated so that the core's own 512
nodes sit in grid columns 0..3 — this keeps the program SPMD-identical
across cores (only input data differs).
"""

import numpy as np
import ml_dtypes

import concourse.bacc as bacc
import concourse.bass as bass
import concourse.mybir as mybir
import concourse.tile as tile
from concourse.bass_utils import run_bass_kernel_spmd

N = 4096            # nodes
HID = 8             # GCN hidden dim
Y = 4096            # output dim
NCORES = 8
NPC = N // NCORES   # 512 nodes per core
WR_DT = mybir.dt.bfloat16
WR_NP = ml_dtypes.bfloat16

F32 = mybir.dt.float32
FP8 = mybir.dt.float8e4
BF16 = mybir.dt.bfloat16
I32 = mybir.dt.int32
AF = mybir.ActivationFunctionType
OP = mybir.AluOpType


def _build_kernel(ct_bf16=False):
    nc = bacc.Bacc("TRN2", target_bir_lowering=False, debug=False,
                   num_devices=NCORES)

    pk_d = nc.dram_tensor("packed", [128, 96], I32, kind="ExternalInput")
    ct_dt = BF16 if ct_bf16 else FP8
    ct_d = nc.dram_tensor("ct", [N, NPC], ct_dt, kind="ExternalInput")
    wb_d = nc.dram_tensor("w1b1", [1, 2 * HID], F32, kind="ExternalInput")
    bias_d = nc.dram_tensor("bias", [1, Y], F32, kind="ExternalInput")
    wr_d = nc.dram_tensor("wr", [8 * NPC, Y], WR_DT, kind="ExternalInput")
    y_d = nc.dram_tensor("y", [1, Y], F32, kind="ExternalOutput")

    with tile.TileContext(nc) as tc:
        with (
            tc.tile_pool(name="small", bufs=1) as sp,
            tc.tile_pool(name="wr", bufs=1) as wp,
            tc.tile_pool(name="psum", bufs=1, space="PSUM") as pp,
        ):
            # ---- small loads ----
            pk_sb = sp.tile([128, 96], I32)
            nc.sync.dma_start(out=pk_sb[:], in_=pk_d[:])
            x_sb = pk_sb[:, 0:32].bitcast(F32)
            inda_sb = pk_sb[:, 32:64]
            indb_sb = pk_sb[:, 64:96]
            wbrow = sp.tile([1, 2 * HID], F32)
            nc.sync.dma_start(out=wbrow[:], in_=wb_d[:])
            w1row = wbrow[:, 0:HID]
            b1row = wbrow[:, HID:2 * HID]
            bias_sb = sp.tile([1, Y], F32)
            nc.sync.dma_start(out=bias_sb[:], in_=bias_d[:])
            # ct in one DMA: SBUF col-slice sc holds ct rows [128sc, 128sc+128)
            ct_sb = sp.tile([128, 32 * NPC], ct_dt)
            nc.sync.dma_start(
                out=ct_sb[:].rearrange("p (sc q) -> p sc q", q=NPC),
                in_=ct_d[:].rearrange("(sc p) q -> p sc q", p=128))

            # ---- deg -> dinv (Rsqrt + one Newton step) ----
            degf_sb = sp.tile([128, 32], F32)
            degi_sb = sp.tile([128, 32], I32)
            nc.vector.tensor_tensor(out=degi_sb[:], in0=indb_sb,
                                    in1=inda_sb, op=OP.subtract)
            nc.vector.tensor_scalar_add(degi_sb[:], degi_sb[:], 1)
            nc.vector.tensor_copy(out=degf_sb[:], in_=degi_sb[:])
            sq_sb = sp.tile([128, 32], F32)
            nc.scalar.activation(sq_sb[:], degf_sb[:], AF.Sqrt)
            y0_sb = sp.tile([128, 32], F32)
            nc.vector.reciprocal(y0_sb[:], sq_sb[:])
            # two Newton steps: y <- y*(1.5 - 0.5*deg*y^2)
            t_sb = sp.tile([128, 32], F32)
            dinv_sb = sp.tile([128, 32], F32)
            for cur, nxt in [(y0_sb, t_sb), (t_sb, dinv_sb)]:
                tmp_sb = sp.tile([128, 32], F32, name=f"nr_{nxt.tensor.name}")
                nc.vector.tensor_tensor(out=tmp_sb[:], in0=cur[:], in1=cur[:],
                                        op=OP.mult)
                nc.vector.tensor_tensor(out=tmp_sb[:], in0=tmp_sb[:],
                                        in1=degf_sb[:], op=OP.mult)
                nc.vector.tensor_scalar(out=tmp_sb[:], in0=tmp_sb[:],
                                        scalar1=-0.5, scalar2=1.5,
                                        op0=OP.mult, op1=OP.add)
                nc.vector.tensor_tensor(out=nxt[:], in0=cur[:], in1=tmp_sb[:],
                                        op=OP.mult)

            # ---- u = x*dinv, split into two bf16 terms ----
            u_sb = sp.tile([128, 32], F32)
            nc.vector.tensor_tensor(out=u_sb[:], in0=x_sb, in1=dinv_sb[:],
                                    op=OP.mult)
            # u = p0 + p1/64 + p2/4096 with each term quantized to fp8e4m3
            u2_sb = sp.tile([128, 96], FP8)
            u2v = u2_sb[:].rearrange("p (c three) -> p c three", three=3)
            res_sb = sp.tile([128, 32], F32)
            cur = u_sb
            for term, scale in enumerate((1.0, 64.0, 4096.0)):
                scl_sb = sp.tile([128, 32], F32, name=f"scl{term}")
                if scale == 1.0:
                    src_ap = cur[:]
                else:
                    nc.vector.tensor_scalar_mul(scl_sb[:], u_sb[:]
                                                if term == 0 else res_sb[:],
                                                scale)
                    src_ap = scl_sb[:]
                nc.vector.tensor_copy(
                    out=u2v[:, :, term:term + 1],
                    in_=src_ap.rearrange("p (c one) -> p c one", one=1))
                if term < 2:
                    back_sb = sp.tile([128, 32], F32, name=f"back{term}")
                    nc.vector.tensor_copy(
                        out=back_sb[:].rearrange("p (c one) -> p c one", one=1),
                        in_=u2v[:, :, term:term + 1])
                    # residual (in original scale): res -= back/scale
                    if scale != 1.0:
                        nc.vector.tensor_scalar_mul(back_sb[:], back_sb[:],
                                                    1.0 / scale)
                    nc.vector.tensor_tensor(
                        out=res_sb[:], in0=(u_sb[:] if term == 0 else res_sb[:]),
                        in1=back_sb[:], op=OP.subtract)

            # ---- agg[d] = sum_s C'[d, s] * u[s]  (4 dblocks x 32 schunks) ----
            agg_ps = [pp.tile([128, 3], F32, name=f"ps{db}") for db in range(4)]
            for db in range(4):
                for sc in range(32):
                    nc.tensor.matmul(
                        out=agg_ps[db][:],
                        lhsT=ct_sb[:, NPC * sc + 128 * db:NPC * sc + 128 * (db + 1)],
                        rhs=u2_sb[:, 3 * sc:3 * sc + 3],
                        start=(sc == 0), stop=(sc == 31))
            # agg = ps[:,0] + ps[:,1]/64 + ps[:,2]/4096
            aggt_sb = sp.tile([128, 12], F32)
            for db in range(4):
                nc.vector.tensor_copy(out=aggt_sb[:, 3 * db:3 * db + 3],
                                      in_=agg_ps[db][:])
            agg_sb = sp.tile([128, 4], F32)
            av = aggt_sb[:].rearrange("p (db three) -> p db three", three=3)
            nc.vector.tensor_scalar_mul(av[:, :, 1:2], av[:, :, 1:2], 1.0 / 64)
            nc.vector.tensor_scalar_mul(av[:, :, 2:3], av[:, :, 2:3], 1.0 / 4096)
            nc.vector.tensor_reduce(out=agg_sb[:],
                                    in_=av,
                                    axis=mybir.AxisListType.X, op=OP.add)

            # s = dinv_own * agg   (own nodes are grid columns 0..3)
            s_sb = sp.tile([128, 4], F32)
            nc.vector.tensor_tensor(out=s_sb[:], in0=agg_sb[:],
                                    in1=dinv_sb[:, 0:4], op=OP.mult)

            # ---- broadcast W1/b1 across partitions via ones-matmul ----
            ones_sb = sp.tile([1, 128], F32)
            nc.vector.memset(ones_sb[:], 1.0)
            wb_ps = pp.tile([128, 2 * HID], F32, name="ps4")
            nc.tensor.matmul(out=wb_ps[:, 0:HID], lhsT=ones_sb[:],
                             rhs=w1row, start=True, stop=True)
            nc.tensor.matmul(out=wb_ps[:, HID:2 * HID], lhsT=ones_sb[:],
                             rhs=b1row, start=True, stop=True)
            wb_sb = sp.tile([128, 2 * HID], F32)
            nc.vector.tensor_copy(out=wb_sb[:], in_=wb_ps[:])

            # ---- h_k = relu(s*W1[k] + b1[k]), laid out [128, 4*8] ----
            h_sb = sp.tile([128, 4 * HID], BF16)
            for kk in range(HID):
                nc.vector.tensor_scalar(
                    out=h_sb[:, 4 * kk:4 * kk + 4], in0=s_sb[:],
                    scalar1=wb_sb[:, kk:kk + 1],
                    scalar2=wb_sb[:, HID + kk:HID + kk + 1],
                    op0=OP.mult, op1=OP.add)
            nc.vector.tensor_scalar_max(h_sb[:], h_sb[:], 0.0)

            # ---- matvec: y[1, 4096] += h_col.T @ Wr_tile ----
            y_ps = [pp.tile([1, 512], F32, name=f"ps{bk}") for bk in range(8)]
            for bk in range(8):
                eng = nc.vector if bk % 2 == 0 else nc.scalar
                if bk % 2 == 0:
                    nc.vector.tensor_copy(out=y_ps[bk][:],
                                          in_=bias_sb[:, 512 * bk:512 * (bk + 1)])
                else:
                    nc.scalar.copy(out=y_ps[bk][:],
                                   in_=bias_sb[:, 512 * bk:512 * (bk + 1)])
            for t in range(32):
                wr_sb = wp.tile([128, Y], WR_DT, name=f"wr{t % 12}")
                nc.sync.dma_start(out=wr_sb[:],
                                  in_=wr_d[128 * t:128 * (t + 1), :])
                kk, c = t // 4, t % 4
                hcol = h_sb[:, 4 * kk + c:4 * kk + c + 1]
                for bk in range(8):
                    nc.tensor.matmul(out=y_ps[bk][:], lhsT=hcol,
                                     rhs=wr_sb[:, 512 * bk:512 * (bk + 1)],
                                     start=False, stop=(t == 31),
                                     skip_group_check=True)

            y_sb = sp.tile([1, Y], F32)
            for bk in range(8):
                if bk % 2 == 0:
                    nc.vector.tensor_copy(out=y_sb[:, 512 * bk:512 * (bk + 1)],
                                          in_=y_ps[bk][:])
                else:
                    nc.scalar.copy(out=y_sb[:, 512 * bk:512 * (bk + 1)],
                                   in_=y_ps[bk][:])
            nc.sync.dma_start(out=y_d[:], in_=y_sb[:])

    nc.compile()
    return nc


_NC_CACHE = {}


def _get_nc(ct_bf16=False):
    if ct_bf16 not in _NC_CACHE:
        _NC_CACHE[ct_bf16] = _build_kernel(ct_bf16)
    return _NC_CACHE[ct_bf16]


def _host_prep(x, edge_index, W1, b1, Wr, br):
    """Graph layout/structure prep only; all FP math runs on device."""
    x = np.ascontiguousarray(x, dtype=np.float32).reshape(N)
    src = np.asarray(edge_index[0], dtype=np.int64)
    dst = np.asarray(edge_index[1], dtype=np.int64)

    indeg = np.bincount(dst, minlength=N)
    indptr = np.zeros(N + 1, dtype=np.int32)
    np.cumsum(indeg, out=indptr[1:])

    W1v = np.ascontiguousarray(W1, dtype=np.float32).reshape(1, HID)
    b1v = np.ascontiguousarray(b1, dtype=np.float32).reshape(1, HID)
    brv = np.ascontiguousarray(br, dtype=np.float32).reshape(1, Y)
    Wr3 = np.ascontiguousarray(Wr, dtype=np.float32).reshape(N, HID, Y)

    in_maps = []
    p = np.arange(128)[:, None]
    for k in range(NCORES):
        rot = (np.arange(32) + 4 * k) % 32          # column rotation
        g = 128 * rot[None, :] + p                  # [128, 32] global node ids

        # dense count matrix for this core's dst rows, + I (self loops)
        mask = (dst >= NPC * k) & (dst < NPC * (k + 1))
        ck = np.zeros((NPC, N), dtype=np.float32)
        np.add.at(ck, (dst[mask] - NPC * k, src[mask]), 1.0)
        ck[np.arange(NPC), NPC * k + np.arange(NPC)] += 1.0
        # counts <= 8 are exact in fp8e4m3; fall back to bf16 otherwise
        ct_bf16 = bool(ck.max() > 8)
        ct_np = ml_dtypes.bfloat16 if ct_bf16 else ml_dtypes.float8_e4m3
        # ct[128*sc + i, q] = C'[q, node(sc, i)]
        srcperm = g.T.reshape(-1)                   # [(sc i)] -> global node
        ct = np.ascontiguousarray(ck[:, srcperm].T).astype(ct_np)

        wr_core = np.ascontiguousarray(
            Wr3[NPC * k:NPC * (k + 1)].transpose(1, 0, 2).reshape(8 * NPC, Y),
            dtype=np.float32).astype(WR_NP)
        packed = np.concatenate([
            x[g].astype(np.float32).view(np.int32),
            indptr[g].astype(np.int32),
            indptr[g + 1].astype(np.int32)], axis=1)
        in_maps.append({
            "_ct_bf16": ct_bf16,
            "packed": np.ascontiguousarray(packed),
            "ct": ct,
            "w1b1": np.concatenate([W1v, b1v], axis=1),
            "bias": brv if k == 0 else np.zeros((1, Y), dtype=np.float32),
            "wr": wr_core,
        })
    return in_maps


def kernel(x, edge_index, W1, b1, Wr, br, _trace=False):
    in_maps = _host_prep(x, edge_index, W1, b1, Wr, br)
    ct_bf16 = any(m.pop("_ct_bf16") for m in in_maps)
    nc = _get_nc(ct_bf16)
    try:
        res = run_bass_kernel_spmd(nc, in_maps, list(range(NCORES)),
                                   trace=_trace)
    except Exception:
        # one retry: recovers from transiently-poisoned device state
        res = run_bass_kernel_spmd(nc, in_maps, list(range(NCORES)),
                                   trace=_trace)
    y = np.zeros(Y, dtype=np.float64)
    for k in range(NCORES):
        y += np.asarray(res.results[k]["y"]).reshape(Y).astype(np.float64)
    out = y.astype(np.float32)
    if _trace:
        return out, res
    return out
